# revision 6
# baseline (speedup 1.0000x reference)
"""Trainium2 Bass kernel for nn_AttentionPointnet (gnn_message_passing).

Data-parallel over batch: 8 cores x 1 sample each (B=8, T=4096).
Per-core program (all on device):
  - KNN: d2 via PE matmul (m = 2 p.pT - sq_s, folded bias row), top-24 per row
    via chunked max8 -> merge rounds -> max_index against full row.
  - 6 attention+resnet blocks: net rows gathered with indirect DMA (512B rows),
    softmax weights on DVE/ACT, weighted K-reduce on DVE, all matmuls on PE in
    feature-major (transposed) layout so weights are the stationary operand.
Everything is fp32.
"""

import sys

sys.path.insert(0, "/opt/trn_rl_repo")

import numpy as np

import concourse.bass as bass
import concourse.bacc as bacc
import concourse.mybir as mybir
import concourse.tile as tile
from concourse.bass import IndirectOffsetOnAxis
from concourse import library_config

F32 = mybir.dt.float32
U32 = mybir.dt.uint32
I16 = mybir.dt.int16
AF = mybir.ActivationFunctionType
ALU = mybir.AluOpType
AX = mybir.AxisListType

B, T, D, H, NB, K, CDIM = 8, 4096, 3, 128, 6, 20, 128
NT = T // 128  # 32 t-tiles
NCHUNK = 32  # 128-wide chunks per row for stage-1 max8


def build_program():
    nc = bacc.Bacc("TRN2", target_bir_lowering=False, debug=False)

    # ---- DRAM I/O ----
    d_lhsT4 = nc.dram_tensor("lhsT4", [4, T], F32, kind="ExternalInput")
    d_rhs4 = nc.dram_tensor("rhs4", [4, T], F32, kind="ExternalInput")
    d_sqcol = nc.dram_tensor("sqcol", [T, 1], F32, kind="ExternalInput")
    d_p = nc.dram_tensor("pdram", [T, 64], F32, kind="ExternalInput")
    d_wpos4 = nc.dram_tensor("wpos4", [4, H], F32, kind="ExternalInput")
    d_wc46 = nc.dram_tensor("wc46", [4, NB], F32, kind="ExternalInput")
    d_score = nc.dram_tensor("scoreconst", [128, 24], F32, kind="ExternalInput")
    d_bias = nc.dram_tensor("biases", [128, 19], F32, kind="ExternalInput")
    d_w0a = nc.dram_tensor("w0a", [NB, H, H], F32, kind="ExternalInput")
    d_w0b = nc.dram_tensor("w0b", [NB, H, H], F32, kind="ExternalInput")
    d_w1 = nc.dram_tensor("w1", [NB, H, H], F32, kind="ExternalInput")
    d_wsa = nc.dram_tensor("wsa", [NB, H, H], F32, kind="ExternalInput")
    d_wsb = nc.dram_tensor("wsb", [NB, H, H], F32, kind="ExternalInput")
    d_wo = nc.dram_tensor("wo", [NB, H, H], F32, kind="ExternalInput")
    d_wcf = nc.dram_tensor("wcf", [H, CDIM], F32, kind="ExternalInput")
    d_ident = nc.dram_tensor("ident", [128, 128], F32, kind="ExternalInput")
    d_brow = nc.dram_tensor("biasrow", [1, 14 * 128], F32, kind="ExternalInput")
    d_out = nc.dram_tensor("outp", [T, CDIM], F32, kind="ExternalOutput")

    with tile.TileContext(nc) as tc:
        with (
            tc.tile_pool(name="const", bufs=1) as constp,
            tc.tile_pool(name="pers", bufs=1) as pers,
            tc.tile_pool(name="dram", bufs=1, space="DRAM") as dramp,
            tc.tile_pool(name="marr", bufs=2) as marrp,
            tc.tile_pool(name="small", bufs=3) as smallp,
            tc.tile_pool(name="pooled", bufs=2) as pooledp,
            tc.tile_pool(name="sbwork", bufs=3) as sbwork,
            tc.tile_pool(name="psd2", bufs=1, space="PSUM") as psd2,
            tc.tile_pool(name="pss", bufs=4, space="PSUM") as pss,
        ):
            nc.gpsimd.load_library(library_config.mlp)
            # ---- load constants into SBUF ----
            brow = constp.tile([1, 14 * 128], F32, tag="brow")
            nc.sync.dma_start(brow[:, :], d_brow.ap())
            lhsT4 = constp.tile([4, T], F32, tag="lhsT4")
            rhs4 = constp.tile([4, T], F32, tag="rhs4")
            sqcol = constp.tile([128, NT], F32, tag="sqcol")  # [t%128? see below]
            wpos4 = constp.tile([4, H], F32, tag="wpos4")
            wc46 = constp.tile([4, NB], F32, tag="wc46")
            scon = constp.tile([128, 24], F32, tag="scon")
            bias = constp.tile([128, 19], F32, tag="bias")
            ident = constp.tile([128, 128], F32, tag="ident")
            onesrow = constp.tile([1, 128], F32, tag="onesrow")
            wcf = constp.tile([128, CDIM], F32, tag="wcf")
            w0a = constp.tile([128, NB, H], F32, tag="w0a")
            w0b = constp.tile([128, NB, H], F32, tag="w0b")
            w1 = constp.tile([128, NB, H], F32, tag="w1")
            wsa = constp.tile([128, NB, H], F32, tag="wsa")
            wsb = constp.tile([128, NB, H], F32, tag="wsb")
            wo = constp.tile([128, NB, H], F32, tag="wo")

            nc.sync.dma_start(lhsT4[:, :], d_lhsT4.ap())
            nc.sync.dma_start(rhs4[:, :], d_rhs4.ap())
            # sqcol dram [T,1] -> sbuf [128, NT]: partition t%128, col t//128
            nc.sync.dma_start(
                sqcol[:, :],
                d_sqcol.ap().rearrange("(n p) one -> p (n one)", p=128),
            )
            nc.sync.dma_start(wpos4[:, :], d_wpos4.ap())
            nc.sync.dma_start(wc46[:, :], d_wc46.ap())
            nc.sync.dma_start(scon[:, :], d_score.ap())
            nc.sync.dma_start(bias[:, :], d_bias.ap())
            nc.sync.dma_start(ident[:, :], d_ident.ap())
            nc.sync.dma_start(onesrow[:, :], d_brow.ap()[0:1, 13 * 128:14 * 128])
            nc.sync.dma_start(wcf[:, :], d_wcf.ap())
            for sb_t, dr in (
                (w0a, d_w0a), (w0b, d_w0b), (w1, d_w1),
                (wsa, d_wsa), (wsb, d_wsb), (wo, d_wo),
            ):
                nc.sync.dma_start(
                    sb_t[:, :, :], dr.ap().rearrange("i hin hout -> hin i hout")
                )

            # ---- persistent activations ----
            netA = pers.tile([128, T], F32, tag="netA")  # feature-major net
            netB = pers.tile([128, T], F32, tag="netB")
            gpos = pers.tile([128, NT * K], U32, tag="gpos")  # neighbor idx
            dis = pers.tile([128, NT * K], F32, tag="dis")  # distances
            ppool = pers.tile([128, NT * K * D], F32, tag="ppool")  # p[idx]
            pwc = pers.tile([128, NT * NB], F32, tag="pwc")  # p.wc456+bc

            netdram = dramp.tile([T, H], F32, tag="netdram")
            idx16 = pers.tile([128, NT * 160], I16, tag="idx16")

            # ================= Phase A: KNN =================
            for j in range(NT):
                lt = lhsT4[:, j * 128:(j + 1) * 128]
                marr = marrp.tile([128, T], F32, tag="marr")
                for half in range(2):
                    ps = psd2.tile([128, 2048], F32, tag="psd2")
                    for q in range(4):
                        s0 = half * 2048 + q * 512
                        nc.tensor.matmul(
                            ps[:, q * 512:(q + 1) * 512],
                            lhsT=lt, rhs=rhs4[:, s0:s0 + 512],
                            start=True, stop=True,
                        )
                    for q in range(4):
                        nc.scalar.copy(
                            marr[:, half * 2048 + q * 512: half * 2048 + (q + 1) * 512],
                            ps[:, q * 512:(q + 1) * 512],
                        )

                # net0 = p @ Wpos + bpos  (feature-major out)
                psn = pss.tile([128, 128], F32, tag="pss")
                nc.tensor.matmul(psn[:, :], lhsT=wpos4[:, :], rhs=lt,
                                 start=True, stop=True)
                nc.scalar.copy(netA[:, j * 128:(j + 1) * 128], psn[:, :])

                # pwc[t, i] = p_t . wc456_i + bc_i   (out [t, 6])
                psw = pss.tile([128, 128], F32, tag="pss")
                nc.tensor.matmul(psw[:, 0:NB], lhsT=lt, rhs=wc46[:, :],
                                 start=True, stop=True)
                nc.scalar.copy(pwc[:, j * NB:(j + 1) * NB], psw[:, 0:NB])

                # ---- top-24 of each row ----
                cand = smallp.tile([128, 256], F32, tag="cand")
                for c in range(NCHUNK):
                    nc.vector.max(cand[:, c * 8:(c + 1) * 8],
                                  marr[:, c * 128:(c + 1) * 128])
                vals = smallp.tile([128, 24], F32, tag="vals")
                wk1 = smallp.tile([128, 256], F32, tag="wk1")
                wk2 = smallp.tile([128, 256], F32, tag="wk2")
                nc.vector.max(vals[:, 0:8], cand[:, :])
                nc.vector.match_replace(wk1[:, :], vals[:, 0:8], cand[:, :], -1e30)
                nc.vector.max(vals[:, 8:16], wk1[:, :])
                nc.vector.match_replace(wk2[:, :], vals[:, 8:16], wk1[:, :], -1e30)
                nc.vector.max(vals[:, 16:24], wk2[:, :])

                gp = gpos[:, j * K:(j + 1) * K]
                for r in range(3):
                    if r < 2:
                        nc.vector.max_index(gp[:, r * 8:(r + 1) * 8],
                                            vals[:, r * 8:(r + 1) * 8], marr[:, :])
                    else:
                        # only ranks 16..19 are needed
                        gtmp = smallp.tile([128, 8], U32, tag="gtmp")
                        nc.vector.max_index(gtmp[:, :], vals[:, 16:24], marr[:, :])
                        nc.vector.tensor_copy(gp[:, 16:20], gtmp[:, 0:4])

                # dis = sqrt(max(sq_t - m, 1e-12))
                d2t = smallp.tile([128, K], F32, tag="d2t")
                nc.vector.tensor_scalar(
                    d2t[:, :], vals[:, 0:K], -1.0, sqcol[:, j:j + 1],
                    op0=ALU.mult, op1=ALU.add,
                )
                nc.vector.tensor_scalar_max(d2t[:, :], d2t[:, :], 1e-12)
                nc.scalar.activation(dis[:, j * K:(j + 1) * K], d2t[:, :], AF.Sqrt)

                # build the 16-wrapped int16 index list for dma_gather
                gi16 = smallp.tile([128, K], I16, tag="gi16")
                nc.vector.tensor_copy(gi16[:, :], gp[:, :])
                gdram = dramp.tile([128, K], I16, tag="gdram")
                nc.sync.dma_start(gdram[:, :], gi16[:, :])
                ix = idx16[:, j * 160:(j + 1) * 160]
                for a in range(8):
                    srca = bass.AP(
                        tensor=gdram[:, :].tensor,
                        offset=gdram[:, :].offset,
                        ap=[[K, 16], [1, K], [16 * K, 8]],
                    )
                    dsta = ix[16 * a:16 * (a + 1), :].rearrange(
                        "q (k g) -> q k g", k=K)
                    nc.sync.dma_start(dsta, srca)
                # gather p rows (padded to 256B) once per tile
                pg = pooledp.tile([128, K, 64], F32, tag="pgath")
                nc.gpsimd.dma_gather(
                    out_ap=pg[:, :, :], in_ap=d_p.ap(), idxs_ap=ix,
                    num_idxs=128 * K, num_idxs_reg=128 * K, elem_size=64,
                    single_packet=False,
                )
                ppj = ppool[:, j * K * D:(j + 1) * K * D]
                nc.vector.tensor_copy(
                    ppj.rearrange("p (k d) -> p k d", d=D), pg[:, :, 0:D]
                )

            # ================= Phase B: blocks =================
            for i in range(NB):
                nin = netA if i % 2 == 0 else netB
                nout = netB if i % 2 == 0 else netA

                # write net_i row-major to DRAM for gathers
                for j in range(NT):
                    pst = pss.tile([128, 128], F32, tag="pss")
                    nc.tensor.transpose(pst[:, :], nin[:, j * 128:(j + 1) * 128],
                                        ident[:, :])
                    nrow = sbwork.tile([128, 128], F32, tag="nrow")
                    nc.scalar.copy(nrow[:, :], pst[:, :])
                    nc.sync.dma_start(
                        netdram[j * 128:(j + 1) * 128, :], nrow[:, :]
                    )

                for j in range(NT):
                    pooled = pooledp.tile([128, K, H], F32, tag="pooled")
                    nc.gpsimd.dma_gather(
                        out_ap=pooled[:, :, :], in_ap=netdram[:, :],
                        idxs_ap=idx16[:, j * 160:(j + 1) * 160],
                        num_idxs=128 * K, num_idxs_reg=128 * K, elem_size=H,
                        single_packet=False,
                    )

                    # ---- scores [t, K] ----
                    sc = smallp.tile([128, K], F32, tag="sc")
                    nc.vector.tensor_scalar(
                        sc[:, :], dis[:, j * K:(j + 1) * K],
                        scon[:, i:i + 1], pwc[:, j * NB + i:j * NB + i + 1],
                        op0=ALU.mult, op1=ALU.add,
                    )
                    ppj = ppool[:, j * K * D:(j + 1) * K * D]
                    ppj3 = ppj.rearrange("p (k d) -> p k d", d=D)
                    for d in range(D):
                        tmpd = smallp.tile([128, K], F32, tag="tmpd")
                        nc.vector.tensor_scalar(
                            tmpd[:, :], ppj3[:, :, d],
                            scon[:, 6 + 3 * i + d:6 + 3 * i + d + 1], None,
                            op0=ALU.mult,
                        )
                        nc.vector.tensor_tensor(sc[:, :], sc[:, :], tmpd[:, :],
                                                op=ALU.add)
                    # softmax over K
                    mx = smallp.tile([128, 1], F32, tag="mx")
                    nc.vector.tensor_reduce(mx[:, :], sc[:, :], axis=AX.X,
                                            op=ALU.max, negate=True)
                    esc = smallp.tile([128, K], F32, tag="esc")
                    nc.scalar.activation(esc[:, :], sc[:, :], AF.Exp,
                                         bias=mx[:, :], scale=1.0)
                    den = smallp.tile([128, 1], F32, tag="den")
                    nc.vector.tensor_reduce(den[:, :], esc[:, :], axis=AX.X,
                                            op=ALU.add)
                    rden = smallp.tile([128, 1], F32, tag="rden")
                    nc.vector.reciprocal(rden[:, :], den[:, :])
                    w20 = smallp.tile([128, K], F32, tag="w20")
                    nc.vector.tensor_scalar(w20[:, :], esc[:, :], rden[:, :], None,
                                            op0=ALU.mult)

                    # ---- weighted sum over K on DVE ----
                    for k in range(K):
                        nc.vector.tensor_scalar(
                            pooled[:, k, :], pooled[:, k, :],
                            w20[:, k:k + 1], None, op0=ALU.mult,
                        )
                    att = sbwork.tile([128, 128], F32, tag="att")
                    pview = pooled[:, :, :].rearrange("p k h -> p h k")
                    nc.vector.tensor_reduce(att[:, :], pview, axis=AX.X, op=ALU.add)

                    # attT = transpose(att)
                    psT = pss.tile([128, 128], F32, tag="pss")
                    nc.tensor.transpose(psT[:, :], att[:, :], ident[:, :])
                    attrawT = sbwork.tile([128, 128], F32, tag="attrawT")
                    nc.scalar.copy(attrawT[:, :], psT[:, :])

                    # att' = Wo.T@attrawT + bo ; relu variant
                    psA = pss.tile([128, 128], F32, tag="pss")
                    nc.tensor.matmul(psA[:, :], lhsT=wo[:, i, :],
                                     rhs=attrawT[:, :], start=True, stop=False)
                    nc.tensor.matmul(psA[:, :],
                                     lhsT=brow[:, (6 + i) * 128:(7 + i) * 128],
                                     rhs=onesrow[:, :], start=False, stop=True)
                    attT = sbwork.tile([128, 128], F32, tag="attT")
                    nc.scalar.copy(attT[:, :], psA[:, :])
                    reluA = sbwork.tile([128, 128], F32, tag="reluA")
                    nc.scalar.activation(reluA[:, :], psA[:, :], AF.Relu,
                                         bias=0.0, scale=1.0)

                    # resnet block (feature-major)
                    ntj = nin[:, j * 128:(j + 1) * 128]
                    reluN = sbwork.tile([128, 128], F32, tag="reluN")
                    nc.scalar.activation(reluN[:, :], ntj, AF.Relu)
                    ps1 = pss.tile([128, 128], F32, tag="pss")
                    nc.tensor.matmul(ps1[:, :], lhsT=w0a[:, i, :],
                                     rhs=reluN[:, :], start=True, stop=False)
                    nc.tensor.matmul(ps1[:, :], lhsT=w0b[:, i, :],
                                     rhs=reluA[:, :], start=False, stop=True)
                    hrelu = sbwork.tile([128, 128], F32, tag="hrelu")
                    nc.scalar.activation(hrelu[:, :], ps1[:, :], AF.Relu,
                                         bias=bias[:, 6 + i:7 + i], scale=1.0)
                    ps2 = pss.tile([128, 128], F32, tag="pss")
                    nc.tensor.matmul(ps2[:, :], lhsT=w1[:, i, :],
                                     rhs=hrelu[:, :], start=True, stop=False)
                    nc.tensor.matmul(ps2[:, :], lhsT=wsa[:, i, :],
                                     rhs=ntj, start=False, stop=False)
                    nc.tensor.matmul(ps2[:, :], lhsT=wsb[:, i, :],
                                     rhs=attT[:, :], start=False, stop=False)
                    if i > 0:
                        # + last_net (= net_i itself)
                        nc.tensor.matmul(ps2[:, :], lhsT=ident[:, :], rhs=ntj,
                                         start=False, stop=False)
                    nc.tensor.matmul(ps2[:, :], lhsT=brow[:, i * 128:(i + 1) * 128],
                                     rhs=onesrow[:, :], start=False, stop=True)
                    nc.scalar.copy(nout[:, j * 128:(j + 1) * 128], ps2[:, :])

            # ================= Final projection =================
            net6 = netA  # after 6 blocks output is back in netA
            for j in range(NT):
                psF = pss.tile([128, 128], F32, tag="pss")
                nc.tensor.matmul(psF[:, :], lhsT=wcf[:, :],
                                 rhs=net6[:, j * 128:(j + 1) * 128],
                                 start=True, stop=False)
                nc.tensor.matmul(psF[:, :], lhsT=brow[:, 12 * 128:13 * 128],
                                 rhs=onesrow[:, :], start=False, stop=True)
                oT = sbwork.tile([128, 128], F32, tag="oT")
                nc.scalar.copy(oT[:, :], psF[:, :])
                psB = pss.tile([128, 128], F32, tag="pss")
                nc.tensor.transpose(psB[:, :], oT[:, :], ident[:, :])
                orow = sbwork.tile([128, 128], F32, tag="orow")
                nc.scalar.copy(orow[:, :], psB[:, :])
                nc.sync.dma_start(d_out.ap()[j * 128:(j + 1) * 128, :], orow[:, :])

    nc.compile()
    return nc


def make_inputs(p_all, weights):
    """Build the per-core input maps. p_all: [B, T, D] f32."""
    w = weights
    shared = {}
    shared["wpos4"] = np.concatenate(
        [w["W_pos"], w["b_pos"][None, :]], axis=0
    ).astype(np.float32)  # [4, H]
    wc = w["att_Wc"][:, :, 0]  # [NB, 7]
    bc = w["att_bc"][:, 0]  # [NB]
    wc46 = np.zeros((4, NB), np.float32)
    wc46[0:3, :] = wc[:, 4:7].T
    wc46[3, :] = bc
    shared["wc46"] = wc46
    scon = np.zeros((128, 24), np.float32)
    scon[:, 0:6] = wc[:, 0][None, :]
    for i in range(NB):
        for d in range(D):
            scon[:, 6 + 3 * i + d] = wc[i, 1 + d]
    shared["scoreconst"] = scon
    biases = np.zeros((128, 19), np.float32)
    for i in range(NB):
        biases[:, i] = w["blk_b1"][i]
        biases[:, 6 + i] = w["blk_b0"][i]
        biases[:, 12 + i] = w["att_bo"][i]
    biases[:, 18] = w["b_c"]
    shared["biases"] = biases
    shared["w0a"] = np.ascontiguousarray(w["blk_W0"][:, :H, :], np.float32)
    shared["w0b"] = np.ascontiguousarray(w["blk_W0"][:, H:, :], np.float32)
    shared["w1"] = np.ascontiguousarray(w["blk_W1"], np.float32)
    shared["wsa"] = np.ascontiguousarray(w["blk_Ws"][:, :H, :], np.float32)
    shared["wsb"] = np.ascontiguousarray(w["blk_Ws"][:, H:, :], np.float32)
    shared["wo"] = np.ascontiguousarray(w["att_Wo"], np.float32)
    shared["wcf"] = np.ascontiguousarray(w["W_c"], np.float32)
    shared["ident"] = np.eye(128, dtype=np.float32)
    brow = np.zeros((1, 14 * 128), np.float32)
    for i in range(NB):
        brow[0, i * 128:(i + 1) * 128] = w["blk_b1"][i]
        brow[0, (6 + i) * 128:(7 + i) * 128] = w["att_bo"][i]
    brow[0, 12 * 128:13 * 128] = w["b_c"]
    brow[0, 13 * 128:14 * 128] = 1.0
    shared["biasrow"] = brow

    in_maps = []
    for c in range(B):
        p = np.asarray(p_all[c], np.float32)  # [T, D]
        sq = (p * p).sum(-1)  # [T]
        m = dict(shared)
        lhsT4 = np.ones((4, T), np.float32)
        lhsT4[0:3, :] = p.T
        m["lhsT4"] = lhsT4
        rhs4 = np.empty((4, T), np.float32)
        rhs4[0:3, :] = 2.0 * p.T
        rhs4[3, :] = -sq
        m["rhs4"] = rhs4
        m["sqcol"] = sq[:, None].astype(np.float32)
        pp = np.zeros((T, 64), np.float32)
        pp[:, :D] = p
        m["pdram"] = pp
        in_maps.append(m)
    return in_maps


_PROGRAM = None


def kernel(**inputs):
    global _PROGRAM
    p_all = np.asarray(inputs["p"], np.float32)
    assert p_all.shape == (B, T, D)
    in_maps = make_inputs(p_all, {k: np.asarray(v) for k, v in inputs.items()})
    if _PROGRAM is None:
        _PROGRAM = build_program()
    from concourse import bass_utils
    res = bass_utils.run_bass_kernel_spmd(
        _PROGRAM, in_maps, core_ids=list(range(B))
    )
    out = np.stack([r["outp"] for r in res.results], axis=0)  # [B, T, CDIM]
    return out.astype(np.float32)


if __name__ == "__main__":
    # smoke: build only
    nc = build_program()
    print("built ok")


# revision 12
# speedup vs baseline: 48.4826x; 48.4826x over previous
"""Trainium2 Bass kernel for nn_AttentionPointnet (gnn_message_passing).

Data-parallel over batch: 8 NeuronCores x 1 sample each (B=8, T=4096).
Per-core program (everything on device):
  - KNN: m = 2 p.p_s - |p_s|^2 on PE (contract dim 4, bias row folded in);
    per-row top-20 of 4096 via per-128-chunk max8 -> merge rounds for the
    rank-20 threshold -> threshold mask -> exclusive-prefix ranks via
    strict-lower-triangular matmul on PE -> GPSIMD local_scatter compacts
    the global indices (s-ascending order; softmax is order-invariant).
  - Gathers: production dma_gather path (GPSIMD mlp library,
    single_packet=False); int16 index lists built on device via a
    DRAM-roundtrip wrap DMA into the 16-partition-wrapped layout.
  - 6 attention+resnet blocks: net stored bf16 row-major in DRAM for 256B-row
    gathers; softmax on DVE+ACT (exp table); weighted K-sum as a chain of
    fused affine_then_add (custom DVE op); all matmuls on PE in feature-major
    layout so weights are the stationary operand; biases folded in as rank-1
    matmuls (HW Identity+bias activation is unreliable); residual added via
    identity matmul into the same PSUM accumulation group.
Compute/activations fp32 except the bf16 gather payload.
"""

import sys

for _p in ("/opt/trn_rl_repo", "/root/.axon_site/_ro/trn_rl_repo"):
    if _p not in sys.path:
        sys.path.append(_p)

import numpy as np

import concourse.bass as bass
import concourse.bacc as bacc
import concourse.mybir as mybir
import concourse.tile as tile
from concourse.bass import IndirectOffsetOnAxis
from concourse import library_config

F32 = mybir.dt.float32
U32 = mybir.dt.uint32
I16 = mybir.dt.int16
BF16 = mybir.dt.bfloat16
AF = mybir.ActivationFunctionType
ALU = mybir.AluOpType
AX = mybir.AxisListType

B, T, D, H, NB, K, CDIM = 8, 4096, 3, 128, 6, 20, 128
NT = T // 128  # 32 t-tiles
NCHUNK = 32  # 128-wide chunks per row for stage-1 max8


def build_program():
    nc = bacc.Bacc("TRN2", target_bir_lowering=False, debug=False)

    # ---- DRAM I/O ----
    d_lhsT4 = nc.dram_tensor("lhsT4", [4, T], F32, kind="ExternalInput")
    d_rhs4 = nc.dram_tensor("rhs4", [4, T], F32, kind="ExternalInput")
    d_sqcol = nc.dram_tensor("sqcol", [T, 1], F32, kind="ExternalInput")
    d_p = nc.dram_tensor("pdram", [T, 64], F32, kind="ExternalInput")
    d_wpos4 = nc.dram_tensor("wpos4", [4, H], F32, kind="ExternalInput")
    d_wc46 = nc.dram_tensor("wc46", [4, NB], F32, kind="ExternalInput")
    d_score = nc.dram_tensor("scoreconst", [128, 24], F32, kind="ExternalInput")
    d_bias = nc.dram_tensor("biases", [128, 19], F32, kind="ExternalInput")
    d_w0a = nc.dram_tensor("w0a", [NB, H, H], F32, kind="ExternalInput")
    d_w0b = nc.dram_tensor("w0b", [NB, H, H], F32, kind="ExternalInput")
    d_w1 = nc.dram_tensor("w1", [NB, H, H], F32, kind="ExternalInput")
    d_wsa = nc.dram_tensor("wsa", [NB, H, H], F32, kind="ExternalInput")
    d_wsb = nc.dram_tensor("wsb", [NB, H, H], F32, kind="ExternalInput")
    d_wo = nc.dram_tensor("wo", [NB, H, H], F32, kind="ExternalInput")
    d_wcf = nc.dram_tensor("wcf", [H, CDIM], F32, kind="ExternalInput")
    d_ident = nc.dram_tensor("ident", [128, 128], F32, kind="ExternalInput")
    d_brow = nc.dram_tensor("biasrow", [1, 14 * 128], F32, kind="ExternalInput")
    d_lt = nc.dram_tensor("ltstrict", [128, 128], F32, kind="ExternalInput")
    d_onesm = nc.dram_tensor("onesm", [128, 128], F32, kind="ExternalInput")
    d_coff = nc.dram_tensor("chunkoff", [128, 256], mybir.dt.uint16, kind="ExternalInput")
    d_ptile = nc.dram_tensor("ptile", [128, 3 * NT], F32, kind="ExternalInput")
    d_out = nc.dram_tensor("outp", [T, CDIM], F32, kind="ExternalOutput")

    with tile.TileContext(nc) as tc:
        with (
            tc.tile_pool(name="const", bufs=1) as constp,
            tc.tile_pool(name="pers", bufs=1) as pers,
            tc.tile_pool(name="dram", bufs=1, space="DRAM") as dramp,
            tc.tile_pool(name="gdram", bufs=NT, space="DRAM") as gdramp,
            tc.tile_pool(name="marr", bufs=2) as marrp,
            tc.tile_pool(name="small", bufs=2) as smallp,
            tc.tile_pool(name="pooled", bufs=2) as pooledp,
            tc.tile_pool(name="sbwork", bufs=3) as sbwork,
            tc.tile_pool(name="psd2", bufs=1, space="PSUM") as psd2,
            tc.tile_pool(name="pss", bufs=4, space="PSUM") as pss,
        ):
            nc.gpsimd.load_library(library_config.local_scatter)
            # ---- load constants into SBUF ----
            brow = constp.tile([1, 14 * 128], F32, tag="brow")
            nc.sync.dma_start(brow[:, :], d_brow.ap())
            lhsT4 = constp.tile([4, T], F32, tag="lhsT4")
            rhs4 = constp.tile([4, T], F32, tag="rhs4")
            sqcol = constp.tile([128, NT], F32, tag="sqcol")  # [t%128? see below]
            wpos4 = constp.tile([4, H], F32, tag="wpos4")
            wc46 = constp.tile([4, NB], F32, tag="wc46")
            scon = constp.tile([128, 24], F32, tag="scon")
            bias = constp.tile([128, 19], F32, tag="bias")
            ident = constp.tile([128, 128], F32, tag="ident")
            onesrow = constp.tile([1, 128], F32, tag="onesrow")
            ltm = constp.tile([128, 128], F32, tag="ltm")
            onesm = constp.tile([128, 128], F32, tag="onesm")
            coff = constp.tile([128, 256], mybir.dt.uint16, tag="coff")
            ptile = constp.tile([128, 3 * NT], F32, tag="ptile")
            wcf = constp.tile([128, CDIM], F32, tag="wcf")
            w0a = constp.tile([128, NB, H], F32, tag="w0a")
            w0b = constp.tile([128, NB, H], F32, tag="w0b")
            w1 = constp.tile([128, NB, H], F32, tag="w1")
            wsa = constp.tile([128, NB, H], F32, tag="wsa")
            wsb = constp.tile([128, NB, H], F32, tag="wsb")
            wo = constp.tile([128, NB, H], F32, tag="wo")

            nc.sync.dma_start(lhsT4[:, :], d_lhsT4.ap())
            nc.sync.dma_start(rhs4[:, :], d_rhs4.ap())
            # sqcol dram [T,1] -> sbuf [128, NT]: partition t%128, col t//128
            nc.sync.dma_start(
                sqcol[:, :],
                d_sqcol.ap().rearrange("(n p) one -> p (n one)", p=128),
            )
            nc.sync.dma_start(wpos4[:, :], d_wpos4.ap())
            nc.sync.dma_start(wc46[:, :], d_wc46.ap())
            nc.sync.dma_start(scon[:, :], d_score.ap())
            nc.sync.dma_start(bias[:, :], d_bias.ap())
            nc.sync.dma_start(ident[:, :], d_ident.ap())
            nc.sync.dma_start(onesrow[:, :], d_brow.ap()[0:1, 13 * 128:14 * 128])
            nc.sync.dma_start(ltm[:, :], d_lt.ap())
            nc.sync.dma_start(onesm[:, :], d_onesm.ap())
            nc.sync.dma_start(coff[:, :], d_coff.ap())
            nc.sync.dma_start(ptile[:, :], d_ptile.ap())
            nc.sync.dma_start(wcf[:, :], d_wcf.ap())
            for sb_t, dr in (
                (w0a, d_w0a), (w0b, d_w0b), (w1, d_w1),
                (wsa, d_wsa), (wsb, d_wsb), (wo, d_wo),
            ):
                nc.sync.dma_start(
                    sb_t[:, :, :], dr.ap().rearrange("i hin hout -> hin i hout")
                )

            # ---- persistent activations ----
            netA = pers.tile([128, T], F32, tag="netA")  # feature-major net
            netB = pers.tile([128, T], F32, tag="netB")
            gpos = pers.tile([128, NT * K], U32, tag="gpos")  # neighbor idx
            dis = pers.tile([128, NT * K], F32, tag="dis")  # distances
            ppool = pers.tile([128, NT * K * D], F32, tag="ppool")  # p[idx]
            pwc = pers.tile([128, NT * NB], F32, tag="pwc")  # p.wc456+bc

            netdram = dramp.tile([T, H], BF16, tag="netdram")
            idx16 = pers.tile([128, NT * 160], I16, tag="idx16")

            # ================= Phase A: KNN =================
            gdram_tiles = []
            for j in range(NT):
                lt = lhsT4[:, j * 128:(j + 1) * 128]
                marr = marrp.tile([128, T], F32, tag="marr")
                for half in range(2):
                    ps = psd2.tile([128, 2048], F32, tag="psd2")
                    for q in range(4):
                        s0 = half * 2048 + q * 512
                        nc.tensor.matmul(
                            ps[:, q * 512:(q + 1) * 512],
                            lhsT=lt, rhs=rhs4[:, s0:s0 + 512],
                            start=True, stop=True,
                        )
                    for q in range(4):
                        nc.scalar.copy(
                            marr[:, half * 2048 + q * 512: half * 2048 + (q + 1) * 512],
                            ps[:, q * 512:(q + 1) * 512],
                        )

                # net0 = p @ Wpos + bpos  (feature-major out)
                psn = pss.tile([128, 128], F32, tag="pss")
                nc.tensor.matmul(psn[:, :], lhsT=wpos4[:, :], rhs=lt,
                                 start=True, stop=True)
                nc.scalar.copy(netA[:, j * 128:(j + 1) * 128], psn[:, :])

                # pwc[t, i] = p_t . wc456_i + bc_i   (out [t, 6])
                psw = pss.tile([128, 128], F32, tag="pss")
                nc.tensor.matmul(psw[:, 0:NB], lhsT=lt, rhs=wc46[:, :],
                                 start=True, stop=True)
                nc.scalar.copy(pwc[:, j * NB:(j + 1) * NB], psw[:, 0:NB])

                # ---- top-24 of each row ----
                cand = smallp.tile([128, 256], F32, tag="cand")
                for c in range(NCHUNK):
                    nc.vector.max(cand[:, c * 8:(c + 1) * 8],
                                  marr[:, c * 128:(c + 1) * 128])
                vals = smallp.tile([128, 24], F32, tag="vals")
                wk1 = smallp.tile([128, 256], F32, tag="wk1")
                wk2 = smallp.tile([128, 256], F32, tag="wk2")
                nc.vector.max(vals[:, 0:8], cand[:, :])
                nc.vector.match_replace(wk1[:, :], vals[:, 0:8], cand[:, :], -1e30)
                nc.vector.max(vals[:, 8:16], wk1[:, :])
                nc.vector.match_replace(wk2[:, :], vals[:, 8:16], wk1[:, :], -1e30)
                nc.vector.max(vals[:, 16:24], wk2[:, :])

                # per-chunk indices -> global candidate index table (f32)
                lidx = smallp.tile([128, 256], mybir.dt.uint16, tag="lidx")
                for c in range(NCHUNK):
                    nc.vector.max_index(lidx[:, c * 8:(c + 1) * 8],
                                        cand[:, c * 8:(c + 1) * 8],
                                        marr[:, c * 128:(c + 1) * 128])
                nc.vector.tensor_tensor(lidx[:, :], lidx[:, :], coff[:, :],
                                        op=ALU.add)

                # selection mask O = cand >= tau (tau = 20th largest)
                Om = smallp.tile([128, 256], F32, tag="Om")
                nc.vector.tensor_scalar(Om[:, :], cand[:, :], vals[:, 19:20],
                                        None, op0=ALU.is_ge)
                # exclusive prefix ranks via PE: pfxT = LT^T.O^T chunks
                psT0 = pss.tile([128, 128], F32, tag="pss")
                nc.tensor.transpose(psT0[:, :], Om[:, 0:128], ident[:, :])
                ot0 = smallp.tile([128, 128], F32, tag="ot0")
                nc.scalar.copy(ot0[:, :], psT0[:, :])
                psT1 = pss.tile([128, 128], F32, tag="pss")
                nc.tensor.transpose(psT1[:, :], Om[:, 128:256], ident[:, :])
                ot1 = smallp.tile([128, 128], F32, tag="ot1")
                nc.scalar.copy(ot1[:, :], psT1[:, :])
                psP0 = pss.tile([128, 128], F32, tag="pss")
                nc.tensor.matmul(psP0[:, :], lhsT=ltm[:, :], rhs=ot0[:, :],
                                 start=True, stop=True)
                pf0 = smallp.tile([128, 128], F32, tag="pf0")
                nc.scalar.copy(pf0[:, :], psP0[:, :])
                psP1 = pss.tile([128, 128], F32, tag="pss")
                nc.tensor.matmul(psP1[:, :], lhsT=onesm[:, :], rhs=ot0[:, :],
                                 start=True, stop=False)
                nc.tensor.matmul(psP1[:, :], lhsT=ltm[:, :], rhs=ot1[:, :],
                                 start=False, stop=True)
                pf1 = smallp.tile([128, 128], F32, tag="pf1")
                nc.scalar.copy(pf1[:, :], psP1[:, :])
                # back-transpose prefix to [t, s]
                pfx = smallp.tile([128, 256], F32, tag="pfx")
                psB0 = pss.tile([128, 128], F32, tag="pss")
                nc.tensor.transpose(psB0[:, :], pf0[:, :], ident[:, :])
                nc.scalar.copy(pfx[:, 0:128], psB0[:, :])
                psB1 = pss.tile([128, 128], F32, tag="pss")
                nc.tensor.transpose(psB1[:, :], pf1[:, :], ident[:, :])
                nc.scalar.copy(pfx[:, 128:256], psB1[:, :])
                # sidx = O ? rank : -1   (as int16)
                om1 = smallp.tile([128, 256], F32, tag="om1")
                nc.vector.tensor_scalar(om1[:, :], Om[:, :], -1.0, None,
                                        op0=ALU.add)
                nc.vector.tensor_tensor(pfx[:, :], pfx[:, :], Om[:, :],
                                        op=ALU.mult)
                sidx = smallp.tile([128, 256], I16, tag="sidx")
                nc.vector.tensor_tensor(sidx[:, :], pfx[:, :], om1[:, :],
                                        op=ALU.add)
                # compact indices to 32 slots (ranks 0..19 used)
                gidx24 = smallp.tile([128, 32], I16, tag="gidx24")
                nc.gpsimd.local_scatter(gidx24[:, :], lidx[:, :], sidx[:, :],
                                        channels=128, num_elems=32, num_idxs=256)
                gi16 = gidx24[:, 0:K]
                gdram = gdramp.tile([128, K], I16, tag="gdram")
                nc.sync.dma_start(gdram[:, :], gi16[:, :])
                gdram_tiles.append(gdram)

            # switch GPSIMD library once, then do all gathers
            nc.gpsimd.load_library(library_config.mlp)
            for j in range(NT):
                gdram = gdram_tiles[j]
                ix = idx16[:, j * 160:(j + 1) * 160]
                for a in range(8):
                    srca = bass.AP(
                        tensor=gdram[:, :].tensor,
                        offset=gdram[:, :].offset,
                        ap=[[K, 16], [1, K], [16 * K, 8]],
                    )
                    dsta = ix[16 * a:16 * (a + 1), :].rearrange(
                        "q (k g) -> q k g", k=K)
                    nc.sync.dma_start(dsta, srca)
                # gather p rows (padded to 256B) once per tile
                pg = pooledp.tile([128, K, 64], F32, tag="pgath")
                nc.gpsimd.dma_gather(
                    out_ap=pg[:, :, :], in_ap=d_p.ap(), idxs_ap=ix,
                    num_idxs=128 * K, num_idxs_reg=128 * K, elem_size=64,
                    single_packet=False,
                )
                ppj = ppool[:, j * K * D:(j + 1) * K * D]
                nc.vector.tensor_copy(
                    ppj.rearrange("p (k d) -> p k d", d=D), pg[:, :, 0:D]
                )
                # dis = |p_t - p_s| from gathered rows
                d2t = smallp.tile([128, K], F32, tag="d2t")
                df = smallp.tile([128, K], F32, tag="df")
                for d in range(D):
                    nc.vector.tensor_scalar(
                        df[:, :], pg[:, :, d], -1.0,
                        ptile[:, 3 * j + d:3 * j + d + 1],
                        op0=ALU.mult, op1=ALU.add,
                    )
                    if d == 0:
                        nc.vector.tensor_tensor(d2t[:, :], df[:, :], df[:, :],
                                                op=ALU.mult)
                    else:
                        nc.vector.tensor_tensor(df[:, :], df[:, :], df[:, :],
                                                op=ALU.mult)
                        nc.vector.tensor_tensor(d2t[:, :], d2t[:, :], df[:, :],
                                                op=ALU.add)
                nc.vector.tensor_scalar_max(d2t[:, :], d2t[:, :], 1e-12)
                nc.scalar.activation(dis[:, j * K:(j + 1) * K], d2t[:, :], AF.Sqrt)

            # ================= Phase B: blocks =================
            for i in range(NB):
                nin = netA if i % 2 == 0 else netB
                nout = netB if i % 2 == 0 else netA

                # write net_i row-major to DRAM for gathers
                for j in range(NT):
                    pst = pss.tile([128, 128], F32, tag="pss")
                    nc.tensor.transpose(pst[:, :], nin[:, j * 128:(j + 1) * 128],
                                        ident[:, :])
                    nrow = sbwork.tile([128, 128], BF16, tag="nrow")
                    nc.scalar.copy(nrow[:, :], pst[:, :])
                    nc.sync.dma_start(
                        netdram[j * 128:(j + 1) * 128, :], nrow[:, :]
                    )

                for j in range(NT):
                    pooled = pooledp.tile([128, K, H], BF16, tag="pooled")
                    nc.gpsimd.dma_gather(
                        out_ap=pooled[:, :, :], in_ap=netdram[:, :],
                        idxs_ap=idx16[:, j * 160:(j + 1) * 160],
                        num_idxs=128 * K, num_idxs_reg=128 * K, elem_size=H,
                        single_packet=False,
                    )

                    # ---- scores [t, K] ----
                    sc = smallp.tile([128, K], F32, tag="sc")
                    nc.vector.tensor_scalar(
                        sc[:, :], dis[:, j * K:(j + 1) * K],
                        scon[:, i:i + 1], pwc[:, j * NB + i:j * NB + i + 1],
                        op0=ALU.mult, op1=ALU.add,
                    )
                    ppj = ppool[:, j * K * D:(j + 1) * K * D]
                    ppj3 = ppj.rearrange("p (k d) -> p k d", d=D)
                    for d in range(D):
                        nc.vector.affine_then_add(
                            sc[:, :], ppj3[:, :, d], sc[:, :],
                            scale=scon[:, 6 + 3 * i + d:6 + 3 * i + d + 1],
                            bias=0.0,
                        )
                    # softmax over K
                    mx = smallp.tile([128, 1], F32, tag="mx")
                    nc.vector.tensor_reduce(mx[:, :], sc[:, :], axis=AX.X,
                                            op=ALU.max, negate=True)
                    esc = smallp.tile([128, K], F32, tag="esc")
                    nc.scalar.activation(esc[:, :], sc[:, :], AF.Exp,
                                         bias=mx[:, :], scale=1.0)
                    den = smallp.tile([128, 1], F32, tag="den")
                    nc.vector.tensor_reduce(den[:, :], esc[:, :], axis=AX.X,
                                            op=ALU.add)
                    rden = smallp.tile([128, 1], F32, tag="rden")
                    nc.vector.reciprocal(rden[:, :], den[:, :])
                    w20 = smallp.tile([128, K], F32, tag="w20")
                    nc.vector.tensor_scalar(w20[:, :], esc[:, :], rden[:, :], None,
                                            op0=ALU.mult)

                    # ---- weighted sum over K: fused multiply-accumulate ----
                    att = sbwork.tile([128, 128], F32, tag="att")
                    nc.vector.tensor_scalar(att[:, :], pooled[:, 0, :],
                                            w20[:, 0:1], None, op0=ALU.mult)
                    for k in range(1, K):
                        nc.vector.affine_then_add(att[:, :], pooled[:, k, :],
                                                  att[:, :],
                                                  scale=w20[:, k:k + 1], bias=0.0)

                    # attT = transpose(att)
                    psT = pss.tile([128, 128], F32, tag="pss")
                    nc.tensor.transpose(psT[:, :], att[:, :], ident[:, :])
                    attrawT = sbwork.tile([128, 128], F32, tag="attrawT")
                    nc.scalar.copy(attrawT[:, :], psT[:, :])

                    # att' = Wo.T@attrawT + bo ; relu variant
                    psA = pss.tile([128, 128], F32, tag="pss")
                    nc.tensor.matmul(psA[:, :], lhsT=wo[:, i, :],
                                     rhs=attrawT[:, :], start=True, stop=False)
                    nc.tensor.matmul(psA[:, :],
                                     lhsT=brow[:, (6 + i) * 128:(7 + i) * 128],
                                     rhs=onesrow[:, :], start=False, stop=True)
                    attT = sbwork.tile([128, 128], F32, tag="attT")
                    nc.scalar.copy(attT[:, :], psA[:, :])
                    reluA = sbwork.tile([128, 128], F32, tag="reluA")
                    nc.scalar.activation(reluA[:, :], psA[:, :], AF.Relu,
                                         bias=0.0, scale=1.0)

                    # resnet block (feature-major)
                    ntj = nin[:, j * 128:(j + 1) * 128]
                    reluN = sbwork.tile([128, 128], F32, tag="reluN")
                    nc.scalar.activation(reluN[:, :], ntj, AF.Relu)
                    ps1 = pss.tile([128, 128], F32, tag="pss")
                    nc.tensor.matmul(ps1[:, :], lhsT=w0a[:, i, :],
                                     rhs=reluN[:, :], start=True, stop=False)
                    nc.tensor.matmul(ps1[:, :], lhsT=w0b[:, i, :],
                                     rhs=reluA[:, :], start=False, stop=True)
                    hrelu = sbwork.tile([128, 128], F32, tag="hrelu")
                    nc.scalar.activation(hrelu[:, :], ps1[:, :], AF.Relu,
                                         bias=bias[:, 6 + i:7 + i], scale=1.0)
                    ps2 = pss.tile([128, 128], F32, tag="pss")
                    nc.tensor.matmul(ps2[:, :], lhsT=w1[:, i, :],
                                     rhs=hrelu[:, :], start=True, stop=False)
                    nc.tensor.matmul(ps2[:, :], lhsT=wsa[:, i, :],
                                     rhs=ntj, start=False, stop=False)
                    nc.tensor.matmul(ps2[:, :], lhsT=wsb[:, i, :],
                                     rhs=attT[:, :], start=False, stop=False)
                    if i > 0:
                        # + last_net (= net_i itself)
                        nc.tensor.matmul(ps2[:, :], lhsT=ident[:, :], rhs=ntj,
                                         start=False, stop=False)
                    nc.tensor.matmul(ps2[:, :], lhsT=brow[:, i * 128:(i + 1) * 128],
                                     rhs=onesrow[:, :], start=False, stop=True)
                    nc.scalar.copy(nout[:, j * 128:(j + 1) * 128], ps2[:, :])

            # ================= Final projection =================
            net6 = netA  # after 6 blocks output is back in netA
            for j in range(NT):
                psF = pss.tile([128, 128], F32, tag="pss")
                nc.tensor.matmul(psF[:, :], lhsT=wcf[:, :],
                                 rhs=net6[:, j * 128:(j + 1) * 128],
                                 start=True, stop=False)
                nc.tensor.matmul(psF[:, :], lhsT=brow[:, 12 * 128:13 * 128],
                                 rhs=onesrow[:, :], start=False, stop=True)
                oT = sbwork.tile([128, 128], F32, tag="oT")
                nc.scalar.copy(oT[:, :], psF[:, :])
                psB = pss.tile([128, 128], F32, tag="pss")
                nc.tensor.transpose(psB[:, :], oT[:, :], ident[:, :])
                orow = sbwork.tile([128, 128], F32, tag="orow")
                nc.scalar.copy(orow[:, :], psB[:, :])
                nc.sync.dma_start(d_out.ap()[j * 128:(j + 1) * 128, :], orow[:, :])

    nc.compile()
    return nc


def make_inputs(p_all, weights):
    """Build the per-core input maps. p_all: [B, T, D] f32."""
    w = weights
    shared = {}
    shared["wpos4"] = np.concatenate(
        [w["W_pos"], w["b_pos"][None, :]], axis=0
    ).astype(np.float32)  # [4, H]
    wc = w["att_Wc"][:, :, 0]  # [NB, 7]
    bc = w["att_bc"][:, 0]  # [NB]
    wc46 = np.zeros((4, NB), np.float32)
    wc46[0:3, :] = wc[:, 4:7].T
    wc46[3, :] = bc
    shared["wc46"] = wc46
    scon = np.zeros((128, 24), np.float32)
    scon[:, 0:6] = wc[:, 0][None, :]
    for i in range(NB):
        for d in range(D):
            scon[:, 6 + 3 * i + d] = wc[i, 1 + d]
    shared["scoreconst"] = scon
    biases = np.zeros((128, 19), np.float32)
    for i in range(NB):
        biases[:, i] = w["blk_b1"][i]
        biases[:, 6 + i] = w["blk_b0"][i]
        biases[:, 12 + i] = w["att_bo"][i]
    biases[:, 18] = w["b_c"]
    shared["biases"] = biases
    shared["w0a"] = np.ascontiguousarray(w["blk_W0"][:, :H, :], np.float32)
    shared["w0b"] = np.ascontiguousarray(w["blk_W0"][:, H:, :], np.float32)
    shared["w1"] = np.ascontiguousarray(w["blk_W1"], np.float32)
    shared["wsa"] = np.ascontiguousarray(w["blk_Ws"][:, :H, :], np.float32)
    shared["wsb"] = np.ascontiguousarray(w["blk_Ws"][:, H:, :], np.float32)
    shared["wo"] = np.ascontiguousarray(w["att_Wo"], np.float32)
    shared["wcf"] = np.ascontiguousarray(w["W_c"], np.float32)
    shared["ident"] = np.eye(128, dtype=np.float32)
    brow = np.zeros((1, 14 * 128), np.float32)
    for i in range(NB):
        brow[0, i * 128:(i + 1) * 128] = w["blk_b1"][i]
        brow[0, (6 + i) * 128:(7 + i) * 128] = w["att_bo"][i]
    brow[0, 12 * 128:13 * 128] = w["b_c"]
    brow[0, 13 * 128:14 * 128] = 1.0
    shared["biasrow"] = brow
    shared["ltstrict"] = np.triu(np.ones((128, 128), np.float32), 1).T.copy()
    shared["onesm"] = np.ones((128, 128), np.float32)
    co = np.zeros((128, 256), np.uint16)
    co[:, :] = (np.arange(256) // 8 * 128)[None, :]
    shared["chunkoff"] = co

    in_maps = []
    for c in range(B):
        p = np.asarray(p_all[c], np.float32)  # [T, D]
        sq = (p * p).sum(-1)  # [T]
        m = dict(shared)
        lhsT4 = np.ones((4, T), np.float32)
        lhsT4[0:3, :] = p.T
        m["lhsT4"] = lhsT4
        rhs4 = np.empty((4, T), np.float32)
        rhs4[0:3, :] = 2.0 * p.T
        rhs4[3, :] = -sq
        m["rhs4"] = rhs4
        m["sqcol"] = sq[:, None].astype(np.float32)
        pp = np.zeros((T, 64), np.float32)
        pp[:, :D] = p
        m["pdram"] = pp
        pt = np.zeros((128, 3 * NT), np.float32)
        for j in range(NT):
            pt[:, 3 * j:3 * j + 3] = p[j * 128:(j + 1) * 128]
        m["ptile"] = pt
        in_maps.append(m)
    return in_maps


_PROGRAM = None


def kernel(**inputs):
    global _PROGRAM
    p_all = np.asarray(inputs["p"], np.float32)
    assert p_all.shape == (B, T, D)
    in_maps = make_inputs(p_all, {k: np.asarray(v) for k, v in inputs.items()})
    if _PROGRAM is None:
        _PROGRAM = build_program()
    from concourse import bass_utils
    res = bass_utils.run_bass_kernel_spmd(
        _PROGRAM, in_maps, core_ids=list(range(B))
    )
    out = np.stack([r["outp"] for r in res.results], axis=0)  # [B, T, CDIM]
    return out.astype(np.float32)


if __name__ == "__main__":
    # smoke: build only
    nc = build_program()
    print("built ok")
